# revision 1
# baseline (speedup 1.0000x reference)
"""GQA attention block (RMSNorm-QK, causal, GQA) on 8 trn2 NeuronCores.

Strategy: sequence sharding, zero collectives. Core c handles batch c//4 and
two causally-balanced query chunks (j and 7-j of 8) of 256 rows each. The host
permutes the key/token axis per core so every core sees its own query tokens
at fixed columns [0:512]; causality becomes per-core *data* (exp-bias columns
+ two constant triangle masks), so one uniform SPMD program serves all cores.

All activations live feature-major ("T layout", [feature, token]) so every
matmul consumes natural layouts with zero on-device transposes except V
(tiny). Scores are computed transposed ([k, q]); softmax needs no max
subtraction because RMS-normalized q,k bound |scores/sqrt(D)| <= sqrt(D).
Partition-dim reductions (RMS sum-of-squares, softmax denominators) are
rank-1 matmuls on the PE; per-token broadcasts are rank-1 matmuls as well.
"""

import math
import numpy as np
from contextlib import ExitStack

import concourse.bass as bass
import concourse.mybir as mybir
import concourse.tile as tile
from concourse import bacc
from concourse.bass_utils import run_bass_kernel_spmd
from concourse.masks import make_identity

F32 = mybir.dt.float32
F32R = mybir.dt.float32r
ADD = mybir.AluOpType.add
MULT = mybir.AluOpType.mult
EXP = mybir.ActivationFunctionType.Exp
SQRT = mybir.ActivationFunctionType.Sqrt
SQUARE = mybir.ActivationFunctionType.Square

EPS = 1e-8
NEG = -50.0  # additive pre-exp mask; exp(-50 + |s|max~11) ~ 1e-17


def full_cfg():
    return dict(B=2, S=2048, E=2048, D=128, G=2)


def derived(cfg):
    B, S, E, D, G = cfg["B"], cfg["S"], cfg["E"], cfg["D"], cfg["G"]
    NH = E // D            # query heads
    ET = E // 128          # 128-row tiles of E (contraction / feature tiles)
    NKT = S // 128         # key tiles
    QPC = S // 4           # query tokens per core (2 chunks)
    CH = S // 8            # chunk size
    TD = CH // 128         # diagonal key-tiles per chunk
    GS = NH // G           # heads per kv group
    assert D == 128 and CH % 128 == 0 and QPC <= 512
    return NH, ET, NKT, QPC, CH, TD, GS


def build_program(cfg):
    B, S, E, D, G = cfg["B"], cfg["S"], cfg["E"], cfg["D"], cfg["G"]
    NH, ET, NKT, QPC, CH, TD, GS = derived(cfg)
    SCALE = 1.0 / math.sqrt(D)
    KC = 512               # key-column chunk width for projections
    NKC = S // KC

    nc = bacc.Bacc()
    xT_d = nc.dram_tensor("xT", [E, S], F32, kind="ExternalInput")
    wq_d = nc.dram_tensor("Wq", [E, E], F32, kind="ExternalInput")
    wk_d = nc.dram_tensor("Wk", [E, G * D], F32, kind="ExternalInput")
    wv_d = nc.dram_tensor("Wv", [E, G * D], F32, kind="ExternalInput")
    wo_d = nc.dram_tensor("Wo", [E, E], F32, kind="ExternalInput")
    bq_d = nc.dram_tensor("bq_t", [128, ET], F32, kind="ExternalInput")
    bk_d = nc.dram_tensor("bk_t", [128, G], F32, kind="ExternalInput")
    bv_d = nc.dram_tensor("bv_t", [128, G], F32, kind="ExternalInput")
    bo_d = nc.dram_tensor("bo_t", [128, ET], F32, kind="ExternalInput")
    gq_d = nc.dram_tensor("gq_r", [1, 128], F32, kind="ExternalInput")
    gk_d = nc.dram_tensor("gk_r", [1, 128], F32, kind="ExternalInput")
    mask_d = nc.dram_tensor("mask", [TD * 128, CH], F32, kind="ExternalInput")
    bcol_d = nc.dram_tensor("bcol", [128, 2 * NKT], F32, kind="ExternalInput")
    ones_d = nc.dram_tensor("ones1", [128, 1], F32, kind="ExternalInput")
    out_d = nc.dram_tensor("outT", [E, QPC], F32, kind="ExternalOutput")

    wq_r = wq_d.rearrange("(t p) c -> p t c", p=128)   # [128, ET, E]
    wk_r = wk_d.rearrange("(t p) c -> p t c", p=128)   # [128, ET, G*D]
    wv_r = wv_d.rearrange("(t p) c -> p t c", p=128)
    wo_r = wo_d.rearrange("(t p) c -> p t c", p=128)

    def r(ap):
        return ap if ap.dtype == F32R else ap.bitcast(F32R)

    with tile.TileContext(nc) as tc, ExitStack() as top:
        consts = top.enter_context(tc.tile_pool(name="consts", bufs=1))
        persist = top.enter_context(tc.tile_pool(name="persist", bufs=1))

        ident = consts.tile([128, 128], F32)
        make_identity(nc, ident)
        ones_col = consts.tile([128, 1], F32R)
        nc.sync.dma_start(out=ones_col, in_=ones_d[:, :].bitcast(F32R))
        ones_row = consts.tile([1, 128], F32)
        nc.vector.memset(ones_row, 1.0)
        eps_t = consts.tile([1, 1], F32)
        nc.vector.memset(eps_t, EPS)
        gq_sb = consts.tile([1, 128], F32)
        nc.sync.dma_start(out=gq_sb, in_=gq_d[:, :])
        gk_sb = consts.tile([1, 128], F32)
        nc.sync.dma_start(out=gk_sb, in_=gk_d[:, :])
        bq_sb = consts.tile([128, ET], F32)
        nc.sync.dma_start(out=bq_sb, in_=bq_d[:, :])
        bk_sb = consts.tile([128, G], F32)
        nc.sync.dma_start(out=bk_sb, in_=bk_d[:, :])
        bv_sb = consts.tile([128, G], F32)
        nc.sync.dma_start(out=bv_sb, in_=bv_d[:, :])
        bo_sb = consts.tile([128, ET], F32)
        nc.sync.dma_start(out=bo_sb, in_=bo_d[:, :])
        bcol_sb = consts.tile([128, 2 * NKT], F32)
        nc.sync.dma_start(out=bcol_sb, in_=bcol_d[:, :])
        mask_sb = []
        for t in range(TD):
            m = consts.tile([128, CH], F32R, tag=f"mask{t}", name=f"mask{t}")
            nc.sync.dma_start(out=m, in_=mask_d[t * 128:(t + 1) * 128, :].bitcast(F32R))
            mask_sb.append(m)

        ktn = [persist.tile([128, S], F32R, tag=f"ktn{g}", name=f"ktn{g}") for g in range(G)]
        vtok = [persist.tile([128, NKT, 128], F32R, tag=f"vtok{g}", name=f"vtok{g}") for g in range(G)]
        qtn = persist.tile([128, NH, QPC], F32R, tag="qtn")

        # ---------------- phase 1+2: projections ------------------------
        with ExitStack() as p12:
            wkvp = p12.enter_context(tc.tile_pool(name="wkv", bufs=1))
            xsp = p12.enter_context(tc.tile_pool(name="xs", bufs=6))
            xqp = p12.enter_context(tc.tile_pool(name="xqp", bufs=1))
            tmp = p12.enter_context(tc.tile_pool(name="tmp12", bufs=3))
            wqp = p12.enter_context(tc.tile_pool(name="wqs", bufs=2))
            pkv = p12.enter_context(tc.tile_pool(name="pkv", bufs=4, space="PSUM"))
            pssq = p12.enter_context(tc.tile_pool(name="pssq", bufs=2, space="PSUM"))
            pbc = p12.enter_context(tc.tile_pool(name="pbc", bufs=2, space="PSUM"))

            wk_sb = wkvp.tile([128, ET, G * D], F32R, tag="wk")
            nc.sync.dma_start(out=wk_sb, in_=wk_r.bitcast(F32R))
            wv_sb = wkvp.tile([128, ET, G * D], F32R, tag="wv")
            nc.sync.dma_start(out=wv_sb, in_=wv_r.bitcast(F32R))

            # one step of deferred post-processing per (kc): list of thunks
            pending = []

            def flush():
                while pending:
                    pending.pop(0)()

            for kc in range(NKC):
                xts = []
                for et in range(ET):
                    xt = xsp.tile([128, KC], F32R, tag="xt")
                    nc.sync.dma_start(
                        out=xt, in_=xT_d[et * 128:(et + 1) * 128,
                                         kc * KC:(kc + 1) * KC].bitcast(F32R))
                    xts.append(xt)
                accs = []
                for ci in range(2 * G):  # K g0, K g1, V g0, V g1
                    acc = pkv.tile([128, KC], F32, tag="pkv", name="acc")
                    accs.append(acc)
                for et in range(ET):
                    for ci in range(2 * G):
                        w_sb = wk_sb if ci < G else wv_sb
                        g = ci % G
                        nc.tensor.matmul(
                            accs[ci],
                            lhsT=r(w_sb[:, et, g * D:(g + 1) * D]),
                            rhs=r(xts[et]),
                            start=(et == 0), stop=(et == ET - 1))
                flush()

                def post_kv(kc=kc, accs=accs):
                    for ci in range(2 * G):
                        g = ci % G
                        is_k = ci < G
                        bsb = bk_sb if is_k else bv_sb
                        vb = tmp.tile([128, KC], F32, tag="vb", name="vb")
                        nc.vector.tensor_scalar(
                            out=vb, in0=accs[ci], scalar1=bsb[:, g:g + 1],
                            scalar2=None, op0=ADD)
                        if is_k:
                            sq = tmp.tile([128, KC], F32R, tag="sq", name="sq")
                            nc.scalar.activation(out=sq, in_=vb, func=SQUARE)
                            ssq = pssq.tile([1, KC], F32, tag="ssq", name="ssq")
                            nc.tensor.matmul(ssq, lhsT=r(ones_col), rhs=r(sq),
                                             start=True, stop=True)
                            rms = tmp.tile([1, KC], F32, tag="rms", name="rms")
                            nc.scalar.activation(out=rms, in_=ssq, func=SQRT,
                                                 scale=1.0 / D, bias=eps_t[:, :])
                            rinv = tmp.tile([1, KC], F32, tag="rinv", name="rinv")
                            nc.vector.reciprocal(out=rinv, in_=rms)
                            bc = pbc.tile([128, KC], F32, tag="bc", name="bc")
                            nc.tensor.matmul(bc, lhsT=gk_sb, rhs=rinv,
                                             start=True, stop=True)
                            nc.vector.tensor_tensor(
                                out=ktn[g][:, kc * KC:(kc + 1) * KC],
                                in0=vb, in1=bc, op=MULT)
                        else:
                            for s in range(KC // 128):
                                vt = pbc.tile([128, 128], F32, tag="bc",
                                              name="vt")
                                nc.tensor.transpose(
                                    vt, in_=vb[:, s * 128:(s + 1) * 128],
                                    identity=ident)
                                kt_i = (kc * KC) // 128 + s
                                nc.scalar.copy(out=vtok[g][:, kt_i, :], in_=vt)
                pending.append(post_kv)
            flush()

            # ---- phase 2: Q projection (query cols are xT[:, 0:QPC]) ----
            xq = []
            for et in range(ET):
                xt = xqp.tile([128, QPC], F32R, tag=f"xq{et}", name=f"xq{et}")
                nc.sync.dma_start(
                    out=xt, in_=xT_d[et * 128:(et + 1) * 128, 0:QPC].bitcast(F32R))
                xq.append(xt)
            for qc in range(NH):
                wq_sb = wqp.tile([128, ET, 128], F32R, tag="wq", name="wq")
                nc.sync.dma_start(
                    out=wq_sb, in_=wq_r[:, :, qc * 128:(qc + 1) * 128].bitcast(F32R))
                acc = pkv.tile([128, QPC], F32, tag="pkv", name="qacc")
                for et in range(ET):
                    nc.tensor.matmul(acc, lhsT=r(wq_sb[:, et, :]),
                                     rhs=r(xq[et]),
                                     start=(et == 0), stop=(et == ET - 1))

                def post_q(qc=qc, acc=acc):
                    vb = tmp.tile([128, QPC], F32, tag="vb", name="qb")
                    nc.vector.tensor_scalar(
                        out=vb, in0=acc, scalar1=bq_sb[:, qc:qc + 1],
                        scalar2=None, op0=ADD)
                    sq = tmp.tile([128, QPC], F32R, tag="sq", name="qsq")
                    nc.scalar.activation(out=sq, in_=vb, func=SQUARE)
                    ssq = pssq.tile([1, QPC], F32, tag="ssq", name="qssq")
                    nc.tensor.matmul(ssq, lhsT=r(ones_col), rhs=r(sq),
                                     start=True, stop=True)
                    rms = tmp.tile([1, QPC], F32, tag="rms", name="qrms")
                    nc.scalar.activation(out=rms, in_=ssq, func=SQRT,
                                         scale=1.0 / D, bias=eps_t[:, :])
                    rinv = tmp.tile([1, QPC], F32, tag="rinv", name="qrinv")
                    nc.vector.reciprocal(out=rinv, in_=rms)
                    bc = pbc.tile([128, QPC], F32, tag="bc", name="qbc")
                    nc.tensor.matmul(bc, lhsT=gq_sb, rhs=rinv,
                                     start=True, stop=True)
                    nc.vector.tensor_tensor(out=qtn[:, qc, :], in0=vb,
                                            in1=bc, op=MULT)
                pending.append(post_q)
                if qc >= 1:
                    pending.pop(0)()
            flush()

        # ---------------- phase 3: attention + phase 4: out proj --------
        with ExitStack() as p34:
            ctxp = p34.enter_context(tc.tile_pool(name="ctxp", bufs=1))
            ctxt = ctxp.tile([128, ET, QPC], F32R, tag="ctxt", name="ctxt")
            ptp = p34.enter_context(tc.tile_pool(name="pt", bufs=4))
            wop = p34.enter_context(tc.tile_pool(name="wos", bufs=3))
            osb = p34.enter_context(tc.tile_pool(name="osb", bufs=3))
            psc = p34.enter_context(tc.tile_pool(name="psc", bufs=2, space="PSUM"))
            pden = p34.enter_context(tc.tile_pool(name="pden", bufs=2, space="PSUM"))
            pcx = p34.enter_context(tc.tile_pool(name="pcx", bufs=3, space="PSUM"))
            pbc2 = p34.enter_context(tc.tile_pool(name="pbc2", bufs=1, space="PSUM"))
            pending2 = []

            def flush2():
                while pending2:
                    pending2.pop(0)()

            for h in range(NH):
                g = h // GS
                den = pden.tile([1, QPC], F32, tag="den", name="den")
                cx = pcx.tile([128, QPC], F32, tag="cx", name="cx")
                for kt in range(NKT):
                    sc = psc.tile([128, QPC], F32, tag="sc", name="sc")
                    nc.tensor.matmul(
                        sc, lhsT=r(ktn[g][:, kt * 128:(kt + 1) * 128]),
                        rhs=r(qtn[:, h, :]), start=True, stop=True)

                    def post_sc(h=h, g=g, kt=kt, sc=sc, den=den, cx=cx):
                        pt = ptp.tile([128, QPC], F32R, tag="pt", name="pt")
                        for half in range(2):
                            nc.scalar.activation(
                                out=pt[:, half * CH:(half + 1) * CH],
                                in_=sc[:, half * CH:(half + 1) * CH],
                                func=EXP, scale=SCALE,
                                bias=bcol_sb[:, half * NKT + kt:
                                             half * NKT + kt + 1])
                        if kt < TD:
                            nc.vector.tensor_tensor(
                                out=pt[:, 0:CH], in0=pt[:, 0:CH],
                                in1=mask_sb[kt], op=MULT)
                        elif kt < 2 * TD:
                            nc.vector.tensor_tensor(
                                out=pt[:, CH:2 * CH], in0=pt[:, CH:2 * CH],
                                in1=mask_sb[kt - TD], op=MULT)
                        nc.tensor.matmul(den, lhsT=r(ones_col), rhs=r(pt),
                                         start=(kt == 0), stop=(kt == NKT - 1))
                        nc.tensor.matmul(cx, lhsT=r(vtok[g][:, kt, :]),
                                         rhs=r(pt),
                                         start=(kt == 0), stop=(kt == NKT - 1))
                    pending2.append(post_sc)
                    if kt >= 1:
                        pending2.pop(0)()

                def post_head(h=h, den=den, cx=cx):
                    rd = ptp.tile([1, QPC], F32, tag="rd", name="rd")
                    nc.vector.reciprocal(out=rd, in_=den)
                    bc2 = pbc2.tile([128, QPC], F32, tag="bc2", name="bc2")
                    nc.tensor.matmul(bc2, lhsT=ones_row, rhs=rd,
                                     start=True, stop=True)
                    bc2s = ptp.tile([128, QPC], F32, tag="bc2s", name="bc2s")
                    nc.vector.tensor_copy(out=bc2s, in_=bc2)
                    nc.vector.tensor_tensor(out=ctxt[:, h, :], in0=cx,
                                            in1=bc2s, op=MULT)
                pending2.append(post_head)
            flush2()

            for c2 in range(ET):
                wo_sb = wop.tile([128, ET, 128], F32R, tag="wo", name="wo")
                nc.sync.dma_start(
                    out=wo_sb, in_=wo_r[:, :, c2 * 128:(c2 + 1) * 128].bitcast(F32R))
                acc = pcx.tile([128, QPC], F32, tag="cx", name="oacc")
                for ct in range(ET):
                    nc.tensor.matmul(acc, lhsT=r(wo_sb[:, ct, :]),
                                     rhs=r(ctxt[:, ct, :]),
                                     start=(ct == 0), stop=(ct == ET - 1))

                def post_o(c2=c2, acc=acc):
                    ot = osb.tile([128, QPC], F32, tag="ot", name="ot")
                    nc.vector.tensor_scalar(
                        out=ot, in0=acc, scalar1=bo_sb[:, c2:c2 + 1],
                        scalar2=None, op0=ADD)
                    nc.sync.dma_start(
                        out=out_d[c2 * 128:(c2 + 1) * 128, :], in_=ot)
                pending2.append(post_o)
                if c2 >= 1:
                    pending2.pop(0)()
            flush2()
    nc.compile()
    return nc


# ---------------------------------------------------------------------------
# host-side sharding
# ---------------------------------------------------------------------------

def core_perm(cfg, j):
    """Permutation of token positions for quarter j: [A | B | c1 | c2 | c3]."""
    S = cfg["S"]
    CH = S // 8
    A = np.arange(CH * j, CH * (j + 1))
    Bc = np.arange(S - CH * (j + 1), S - CH * j)
    rest = np.setdiff1d(np.arange(S), np.concatenate([A, Bc]))
    c1 = rest[rest < CH * j]                                # before A
    c3 = rest[rest >= S - CH * j]                           # after B
    c2 = rest[(rest >= CH * j) & (rest < S - CH * j)]       # middle
    perm = np.concatenate([A, Bc, c1, c2, c3])
    assert perm.shape == (S,)
    return perm


def core_biascol(cfg, j):
    """[128, 2*NKT] additive exp biases (0 keep / NEG drop) per k-tile."""
    S = cfg["S"]
    NKT = S // 128
    CH = S // 8
    TD = CH // 128
    bc = np.zeros((128, 2 * NKT), np.float32)
    for kt in range(NKT):
        lo = kt * 128
        # half A (queries = chunk j): valid keys are perm cols [0,CH) (tri,
        # handled by mask => bias 0) and c1 block [2CH, 2CH + CH*j)
        validA = (lo < CH) or (2 * CH <= lo < 2 * CH + CH * j)
        # half B: valid keys: A cols [0,CH), own tri [CH,2CH), c1+c2 block
        # [2CH, 2CH + CH*j + (S - 2CH - 2CH*j)) = [2CH, S - CH*j)
        validB = (lo < 2 * CH) or (2 * CH <= lo < S - CH * j)
        bc[:, kt] = 0.0 if validA else NEG
        bc[:, NKT + kt] = 0.0 if validB else NEG
    return bc


def tri_masks(cfg):
    S = cfg["S"]
    CH = S // 8
    TD = CH // 128
    m = np.zeros((TD * 128, CH), np.float32)
    for t in range(TD):
        kk = np.arange(128)[:, None] + t * 128
        qq = np.arange(CH)[None, :]
        m[t * 128:(t + 1) * 128, :] = (kk <= qq).astype(np.float32)
    return m


def make_in_maps(cfg, inputs):
    """Build the 8 per-core input dicts. Returns (in_maps, perms)."""
    B, S, E, D, G = cfg["B"], cfg["S"], cfg["E"], cfg["D"], cfg["G"]
    NH, ET, NKT, QPC, CH, TD, GS = derived(cfg)
    x = np.asarray(inputs["x"], np.float32)
    shared = dict(
        Wq=np.ascontiguousarray(inputs["Wq"], np.float32),
        Wk=np.ascontiguousarray(inputs["Wk"], np.float32),
        Wv=np.ascontiguousarray(inputs["Wv"], np.float32),
        Wo=np.ascontiguousarray(inputs["Wo"], np.float32),
        bq_t=np.ascontiguousarray(
            np.asarray(inputs["bq"], np.float32).reshape(ET, 128).T),
        bk_t=np.ascontiguousarray(
            np.asarray(inputs["bk"], np.float32).reshape(G, 128).T),
        bv_t=np.ascontiguousarray(
            np.asarray(inputs["bv"], np.float32).reshape(G, 128).T),
        bo_t=np.ascontiguousarray(
            np.asarray(inputs["bo"], np.float32).reshape(ET, 128).T),
        gq_r=np.ascontiguousarray(
            np.asarray(inputs["gamma_q"], np.float32).reshape(1, 128)),
        gk_r=np.ascontiguousarray(
            np.asarray(inputs["gamma_k"], np.float32).reshape(1, 128)),
        mask=tri_masks(cfg),
        ones1=np.ones((128, 1), np.float32),
    )
    in_maps, perms = [], []
    for c in range(8):
        b, j = c // 4, c % 4
        perm = core_perm(cfg, j)
        xt = np.ascontiguousarray(x[b].T[:, perm])  # [E, S] permuted cols
        m = dict(shared)
        m["xT"] = xt
        m["bcol"] = core_biascol(cfg, j)
        in_maps.append(m)
        perms.append(perm)
    return in_maps, perms


def assemble(cfg, results, perms):
    B, S, E = cfg["B"], cfg["S"], cfg["E"]
    QPC = S // 4
    out = np.empty((B, S, E), np.float32)
    for c in range(8):
        b = c // 4
        out[b, perms[c][:QPC], :] = results[c]["outT"].T
    return out


_CACHE = {}


def kernel(**inputs):
    cfg = full_cfg()
    if "nc" not in _CACHE:
        _CACHE["nc"] = build_program(cfg)
    nc = _CACHE["nc"]
    in_maps, perms = make_in_maps(cfg, inputs)
    res = run_bass_kernel_spmd(nc, in_maps, list(range(8)))
    return assemble(cfg, res.results, perms)



# revision 31
# speedup vs baseline: 1.1396x; 1.1396x over previous
"""GQA attention block (RMSNorm-QK, causal, GQA) on 8 trn2 NeuronCores.

Strategy: strided sequence sharding, zero collectives. Core c handles batch
c//4 and queries at positions j::4 (j = c%4) of that batch, keys in natural
order. With stride-4 queries sorted ascending, causality is *uniform* across
cores: for key tile kt, exactly the query columns [32*kt, 512) are (at least
partially) valid, independent of j. Scores/exp/den/context matmuls are all
sliced to that suffix (~47% less attention work than full S), and the only
j-dependent data is a single [128, 32] diagonal band mask from the host.

All activations are feature-major ([feature, token]); V is produced directly
in [token, d] layout by swapping matmul operands (x tile stationary), so the
kernel has zero on-device transposes. Softmax needs no max subtraction
(RMS-normalized q,k bound |scores|/sqrt(D) <= sqrt(D)). Per-token softmax
denominators are rank-1 PE matmuls; 1/den uses the fast custom-DVE
reciprocal; RMS rsqrt runs on the ACT engine (one table set per phase).
"""

import math
import os
import numpy as np
from contextlib import ExitStack

import concourse.bass as bass
import concourse.mybir as mybir
import concourse.tile as tile
from concourse import bacc
from concourse.bass_utils import run_bass_kernel_spmd

F32 = mybir.dt.float32
F32R = mybir.dt.float32r
ADD = mybir.AluOpType.add
MULT = mybir.AluOpType.mult
EXP = mybir.ActivationFunctionType.Exp
SQRT = mybir.ActivationFunctionType.Sqrt

EPS = 1e-8


def full_cfg():
    return dict(B=2, S=2048, E=2048, D=128, G=2)


def derived(cfg):
    B, S, E, D, G = cfg["B"], cfg["S"], cfg["E"], cfg["D"], cfg["G"]
    NH = E // D            # query heads (16)
    ET = E // 128          # 128-row tiles of E (16)
    NKT = S // 128         # key tiles (16)
    QPC = S // 4           # query tokens per core (512)
    GS = NH // G           # heads per kv group (8)
    assert D == 128 and QPC == 512
    return NH, ET, NKT, QPC, GS


def build_program(cfg):
    B, S, E, D, G = cfg["B"], cfg["S"], cfg["E"], cfg["D"], cfg["G"]
    NH, ET, NKT, QPC, GS = derived(cfg)
    SCALE = 1.0 / math.sqrt(D)
    KC = 512               # token-column chunk width for K/V projections
    NKC = S // KC          # 4
    GRP = 3                # key tiles per exp group (3 PSUM banks)
    NGRP = (NKT + GRP - 1) // GRP  # 6 (last group has 1)
    sliced = os.environ.get("KERNEL_NO_SLICE", "0") != "1"

    def co(kt):            # first computed query column for key tile kt
        return 32 * kt if sliced else 0

    nc = bacc.Bacc()
    xT_d = nc.dram_tensor("xT", [E, S], F32, kind="ExternalInput")
    xq_d = nc.dram_tensor("xq", [E, QPC], F32, kind="ExternalInput")
    wq_d = nc.dram_tensor("Wq", [E, E], F32, kind="ExternalInput")
    wk_d = nc.dram_tensor("Wk", [E, G * D], F32, kind="ExternalInput")
    wv_d = nc.dram_tensor("Wv", [E, G * D], F32, kind="ExternalInput")
    wo_d = nc.dram_tensor("Wo", [E, E], F32, kind="ExternalInput")
    bq_d = nc.dram_tensor("bq_t", [128, ET], F32, kind="ExternalInput")
    bk_d = nc.dram_tensor("bk_t", [128, G], F32, kind="ExternalInput")
    bv_d = nc.dram_tensor("bv_b", [128, G * D], F32, kind="ExternalInput")
    bo_d = nc.dram_tensor("bo_t", [128, ET], F32, kind="ExternalInput")
    gq_d = nc.dram_tensor("gq_r", [1, 128], F32, kind="ExternalInput")
    gk_d = nc.dram_tensor("gk_r", [1, 128], F32, kind="ExternalInput")
    mask_d = nc.dram_tensor("mask32", [128, 32], F32, kind="ExternalInput")
    ones_d = nc.dram_tensor("ones1", [128, 1], F32, kind="ExternalInput")
    out_d = nc.dram_tensor("outT", [E, QPC], F32, kind="ExternalOutput")
    debug = os.environ.get("KERNEL_DEBUG_DEN", "0") == "1"
    if debug:
        dbgden_d = nc.dram_tensor("dbgden", [NH, QPC], F32, kind="ExternalOutput")
        dbgcx_d = nc.dram_tensor("dbgcx", [128, QPC], F32, kind="ExternalOutput")
        dbgq_d = nc.dram_tensor("dbgq", [128, QPC], F32, kind="ExternalOutput")
        dbgk_d = nc.dram_tensor("dbgk", [128, S], F32, kind="ExternalOutput")
        dbgv_d = nc.dram_tensor("dbgv", [128, NKT * 128], F32, kind="ExternalOutput")
        dbgsc_d = nc.dram_tensor("dbgsc", [NKT * 128, QPC], F32, kind="ExternalOutput")
        dbgpt_d = nc.dram_tensor("dbgpt", [NKT * 128, QPC], F32, kind="ExternalOutput")

    wq_r = wq_d.rearrange("(t p) c -> p t c", p=128)   # [128, ET, E]
    wk_r = wk_d.rearrange("(t p) c -> p t c", p=128)   # [128, ET, G*D]
    wv_r = wv_d.rearrange("(t p) c -> p t c", p=128)
    wo_r = wo_d.rearrange("(t p) c -> p t c", p=128)

    def r(ap):
        return ap if ap.dtype == F32R else ap.bitcast(F32R)

    with tile.TileContext(nc) as tc, ExitStack() as top:
        consts = top.enter_context(tc.tile_pool(name="consts", bufs=1))
        persist = top.enter_context(tc.tile_pool(name="persist", bufs=1))

        ones_col = consts.tile([128, 1], F32R)
        nc.sync.dma_start(out=ones_col, in_=ones_d[:, :].bitcast(F32R))
        ones_row = consts.tile([1, 128], F32)
        nc.vector.memset(ones_row, 1.0)
        eps_t = consts.tile([1, 1], F32)
        nc.vector.memset(eps_t, EPS)
        gq_sb = consts.tile([1, 128], F32)
        nc.sync.dma_start(out=gq_sb, in_=gq_d[:, :])
        gk_sb = consts.tile([1, 128], F32)
        nc.sync.dma_start(out=gk_sb, in_=gk_d[:, :])
        bq_sb = consts.tile([128, ET], F32)
        nc.sync.dma_start(out=bq_sb, in_=bq_d[:, :])
        bk_sb = consts.tile([128, G], F32)
        nc.sync.dma_start(out=bk_sb, in_=bk_d[:, :])
        bv_sb = consts.tile([128, G * D], F32)
        nc.sync.dma_start(out=bv_sb, in_=bv_d[:, :])
        bo_sb = consts.tile([128, ET], F32)
        nc.sync.dma_start(out=bo_sb, in_=bo_d[:, :])
        mask_sb = consts.tile([128, 32], F32R)
        nc.sync.dma_start(out=mask_sb, in_=mask_d[:, :].bitcast(F32R))

        ktn = [persist.tile([128, S], F32R, tag=f"ktn{g}", name=f"ktn{g}") for g in range(G)]
        vtok = [persist.tile([128, NKT, 128], F32R, tag=f"vtok{g}", name=f"vtok{g}") for g in range(G)]
        qtn = persist.tile([128, NH, QPC], F32R, tag="qtn")

        # ---------------- phase 1+2: projections ------------------------
        with ExitStack() as p12:
            wkvp = p12.enter_context(tc.tile_pool(name="wkv", bufs=1))
            xsp = p12.enter_context(tc.tile_pool(name="xs", bufs=10))
            xqp = p12.enter_context(tc.tile_pool(name="xqp", bufs=1))
            tmp = p12.enter_context(tc.tile_pool(name="tmp12", bufs=3))
            wqp = p12.enter_context(tc.tile_pool(name="wqs", bufs=3))
            pkv = p12.enter_context(tc.tile_pool(name="pkv", bufs=2, space="PSUM"))
            pv = p12.enter_context(tc.tile_pool(name="pv", bufs=4, space="PSUM"))
            pssq = p12.enter_context(tc.tile_pool(name="pssq", bufs=1, space="PSUM"))
            pbc = p12.enter_context(tc.tile_pool(name="pbc", bufs=1, space="PSUM"))

            wk_sb = wkvp.tile([128, ET, G * D], F32R, tag="wk")
            wv_sb = wkvp.tile([128, ET, G * D], F32R, tag="wv")
            # per-et slices so the first matmuls can start early
            for et in range(ET):
                nc.sync.dma_start(out=wk_sb[:, et, :], in_=wk_r[:, et, :].bitcast(F32R))
                nc.sync.dma_start(out=wv_sb[:, et, :], in_=wv_r[:, et, :].bitcast(F32R))

            pending = []

            def flush():
                while pending:
                    pending.pop(0)()

            for kc in range(NKC):
                xts = []
                for et in range(ET):
                    xt = xsp.tile([128, KC], F32R, tag="xt")
                    nc.sync.dma_start(
                        out=xt, in_=xT_d[et * 128:(et + 1) * 128,
                                         kc * KC:(kc + 1) * KC].bitcast(F32R))
                    xts.append(xt)
                # K projection: [d, tok] layout
                acck = []
                for g in range(G):
                    acc = pkv.tile([128, KC], F32, tag="pkv", name="acck")
                    acck.append(acc)
                for et in range(ET):
                    for g in range(G):
                        nc.tensor.matmul(
                            acck[g],
                            lhsT=r(wk_sb[:, et, g * D:(g + 1) * D]),
                            rhs=r(xts[et]),
                            start=(et == 0), stop=(et == ET - 1))
                # V projection: [tok, d] layout (x tile stationary), bias folded
                accv = []
                for s in range(KC // 128):
                    acc = pv.tile([128, G * D], F32, tag="pv", name="accv")
                    accv.append(acc)
                    for et in range(ET):
                        nc.tensor.matmul(
                            acc,
                            lhsT=r(xts[et][:, s * 128:(s + 1) * 128]),
                            rhs=r(wv_sb[:, et, :]),
                            start=(et == 0), stop=(et == ET - 1))
                flush()

                def post_kv(kc=kc, acck=acck, accv=accv):
                    for g in range(G):
                        vb = tmp.tile([128, KC], F32, tag="vb", name="vb")
                        nc.vector.tensor_scalar(
                            out=vb, in0=acck[g], scalar1=bk_sb[:, g:g + 1],
                            scalar2=None, op0=ADD)
                        sq = tmp.tile([128, KC], F32R, tag="sq", name="sq")
                        nc.vector.tensor_tensor(out=sq, in0=vb, in1=vb, op=MULT)
                        ssq = pssq.tile([1, KC], F32, tag="ssq", name="ssq")
                        nc.tensor.matmul(ssq, lhsT=ones_col, rhs=r(sq),
                                         start=True, stop=True)
                        rms = tmp.tile([1, KC], F32, tag="rms", name="rms")
                        nc.scalar.activation(out=rms, in_=ssq, func=SQRT,
                                             scale=1.0 / D, bias=eps_t[:, :])
                        rinv = tmp.tile([1, KC], F32, tag="rinv", name="rinv")
                        nc.vector.reciprocal_approx_fast(out=rinv, in_=rms)
                        bc = pbc.tile([128, KC], F32, tag="bc", name="bc")
                        nc.tensor.matmul(bc, lhsT=gk_sb, rhs=rinv,
                                         start=True, stop=True)
                        nc.vector.tensor_tensor(
                            out=ktn[g][:, kc * KC:(kc + 1) * KC],
                            in0=vb, in1=bc, op=MULT)
                    for s in range(KC // 128):
                        kt = kc * (KC // 128) + s
                        for g in range(G):
                            nc.vector.tensor_tensor(
                                out=vtok[g][:, kt, :],
                                in0=accv[s][:, g * D:(g + 1) * D],
                                in1=bv_sb[:, g * D:(g + 1) * D], op=ADD)
                pending.append(post_kv)
            flush()

            # ---- phase 2: Q projection from host-gathered strided cols ----
            xq = []
            for et in range(ET):
                xt = xqp.tile([128, QPC], F32R, tag=f"xq{et}", name=f"xq{et}")
                nc.sync.dma_start(
                    out=xt, in_=xq_d[et * 128:(et + 1) * 128, :].bitcast(F32R))
                xq.append(xt)
            for qc in range(NH):
                wq_sb = wqp.tile([128, ET, 128], F32R, tag="wq", name="wq")
                nc.sync.dma_start(
                    out=wq_sb, in_=wq_r[:, :, qc * 128:(qc + 1) * 128].bitcast(F32R))
                acc = pkv.tile([128, QPC], F32, tag="pkv", name="qacc")
                for et in range(ET):
                    nc.tensor.matmul(acc, lhsT=r(wq_sb[:, et, :]),
                                     rhs=r(xq[et]),
                                     start=(et == 0), stop=(et == ET - 1))

                def post_q(qc=qc, acc=acc):
                    vb = tmp.tile([128, QPC], F32, tag="vb", name="qb")
                    nc.vector.tensor_scalar(
                        out=vb, in0=acc, scalar1=bq_sb[:, qc:qc + 1],
                        scalar2=None, op0=ADD)
                    sq = tmp.tile([128, QPC], F32R, tag="sq", name="qsq")
                    nc.vector.tensor_tensor(out=sq, in0=vb, in1=vb, op=MULT)
                    ssq = pssq.tile([1, QPC], F32, tag="ssq", name="qssq")
                    nc.tensor.matmul(ssq, lhsT=ones_col, rhs=r(sq),
                                     start=True, stop=True)
                    rms = tmp.tile([1, QPC], F32, tag="rms", name="qrms")
                    nc.scalar.activation(out=rms, in_=ssq, func=SQRT,
                                         scale=1.0 / D, bias=eps_t[:, :])
                    rinv = tmp.tile([1, QPC], F32, tag="rinv", name="qrinv")
                    nc.vector.reciprocal_approx_fast(out=rinv, in_=rms)
                    bc = pbc.tile([128, QPC], F32, tag="bc", name="qbc")
                    nc.tensor.matmul(bc, lhsT=gq_sb, rhs=rinv,
                                     start=True, stop=True)
                    nc.vector.tensor_tensor(out=qtn[:, qc, :], in0=vb,
                                            in1=bc, op=MULT)
                pending.append(post_q)
                if qc >= 1:
                    pending.pop(0)()
            flush()

        # ---------------- phase 3: attention + phase 4: out proj --------
        with ExitStack() as p34:
            ctxp = p34.enter_context(tc.tile_pool(name="ctxp", bufs=1))
            ctxt = ctxp.tile([128, ET, QPC], F32R, tag="ctxt", name="ctxt")
            ptp = p34.enter_context(tc.tile_pool(name="pt", bufs=1))
            smal = p34.enter_context(tc.tile_pool(name="smal", bufs=1))
            wop = p34.enter_context(tc.tile_pool(name="wos", bufs=3))
            osb = p34.enter_context(tc.tile_pool(name="osb", bufs=3))
            psc = p34.enter_context(tc.tile_pool(name="psc", bufs=1, space="PSUM"))
            pcx = p34.enter_context(tc.tile_pool(name="pcx", bufs=1, space="PSUM"))
            pdn = p34.enter_context(tc.tile_pool(name="pdn", bufs=1, space="PSUM"))

            cx = pcx.tile([128, QPC], F32, tag="cx", name="cx")
            dnb = pdn.tile([128, QPC], F32, tag="dnb", name="dnb")
            rd = smal.tile([1, QPC], F32, tag="rd", name="rd")
            bc2s = smal.tile([128, QPC], F32, tag="bc2s", name="bc2s")

            pending2 = []

            def flush2():
                while pending2:
                    pending2.pop(0)()

            if debug:
                nc.sync.dma_start(out=dbgq_d[:, :], in_=qtn[:, 0, :].bitcast(F32))
                nc.sync.dma_start(out=dbgk_d[:, :], in_=ktn[0][:, :].bitcast(F32))
                nc.sync.dma_start(
                    out=dbgv_d[:, :],
                    in_=vtok[0][:, :, :].bitcast(F32).rearrange("p a b -> p (a b)"))
            for h in range(NH):
                g_kv = h // GS
                for grp in range(NGRP):
                    kts = list(range(grp * GRP, min((grp + 1) * GRP, NKT)))
                    c0g = co(kts[0])
                    # fresh pool tiles per group: rotation inserts the WAR
                    # deps that make deferred emission safe
                    sct = psc.tile([128, GRP, QPC], F32, tag="sc", name="sct")
                    ptt = ptp.tile([128, GRP, QPC], F32R, tag="pt", name="ptt")
                    for i, kt in enumerate(kts):
                        nc.tensor.matmul(
                            sct[:, i, c0g:QPC],
                            lhsT=r(ktn[g_kv][:, kt * 128:(kt + 1) * 128]),
                            rhs=r(qtn[:, h, c0g:QPC]),
                            start=True, stop=True)

                    def post_grp(h=h, g_kv=g_kv, kts=kts, c0g=c0g,
                                 sct=sct, ptt=ptt):
                        n = len(kts)
                        if debug and h == 0:
                            for i, kt in enumerate(kts):
                                dsc = smal.tile([128, QPC], F32, tag="dsc",
                                                name="dsc", bufs=2)
                                nc.vector.tensor_copy(
                                    out=dsc[:, c0g:], in_=sct[:, i, c0g:])
                                nc.sync.dma_start(
                                    out=dbgsc_d[kt * 128:(kt + 1) * 128, c0g:],
                                    in_=dsc[:, c0g:])
                        nc.scalar.activation(
                            out=ptt[:, 0:n, c0g:QPC], in_=sct[:, 0:n, c0g:QPC],
                            func=EXP, scale=SCALE)
                        for i, kt in enumerate(kts):
                            nc.vector.tensor_tensor(
                                out=ptt[:, i, 32 * kt:32 * kt + 32],
                                in0=ptt[:, i, 32 * kt:32 * kt + 32],
                                in1=mask_sb, op=MULT)
                            if debug and h == 0:
                                nc.sync.dma_start(
                                    out=dbgpt_d[kt * 128:(kt + 1) * 128, c0g:],
                                    in_=ptt[:, i, c0g:].bitcast(F32))
                            nc.tensor.matmul(
                                dnb[0:1, co(kt):QPC], lhsT=ones_col,
                                rhs=r(ptt[:, i, co(kt):QPC]),
                                start=(kt == 0), stop=(kt == NKT - 1))
                            nc.tensor.matmul(
                                cx[:, co(kt):QPC],
                                lhsT=r(vtok[g_kv][:, kt, :]),
                                rhs=r(ptt[:, i, co(kt):QPC]),
                                start=(kt == 0), stop=(kt == NKT - 1))
                    pending2.append(post_grp)
                    while len(pending2) > 2:
                        pending2.pop(0)()

                def post_head(h=h):
                    if debug:
                        dsb = smal.tile([1, QPC], F32, tag=f"dbg{h}", name=f"dbg{h}")
                        nc.vector.tensor_copy(out=dsb, in_=dnb[0:1, :])
                        nc.sync.dma_start(out=dbgden_d[h:h + 1, :], in_=dsb)
                        if h == 0:
                            csb = smal.tile([128, QPC], F32, tag="dbgc", name="dbgc")
                            nc.vector.tensor_copy(out=csb, in_=cx)
                            nc.sync.dma_start(out=dbgcx_d[:, :], in_=csb)
                    nc.vector.reciprocal_approx_fast(out=rd, in_=dnb[0:1, :])
                    nc.tensor.matmul(dnb, lhsT=ones_row, rhs=rd,
                                     start=True, stop=True)
                    nc.vector.tensor_copy(out=bc2s, in_=dnb)
                    nc.vector.tensor_tensor(out=ctxt[:, h, :], in0=cx,
                                            in1=bc2s, op=MULT)
                pending2.append(post_head)
            flush2()

            for c2 in range(ET):
                wo_sb = wop.tile([128, ET, 128], F32R, tag="wo", name="wo")
                nc.sync.dma_start(
                    out=wo_sb, in_=wo_r[:, :, c2 * 128:(c2 + 1) * 128].bitcast(F32R))
                acc = pcx.tile([128, QPC], F32, tag="cx", name="oacc")
                for ct in range(ET):
                    nc.tensor.matmul(acc, lhsT=r(wo_sb[:, ct, :]),
                                     rhs=r(ctxt[:, ct, :]),
                                     start=(ct == 0), stop=(ct == ET - 1))

                def post_o(c2=c2, acc=acc):
                    ot = osb.tile([128, QPC], F32, tag="ot", name="ot")
                    nc.vector.tensor_scalar(
                        out=ot, in0=acc, scalar1=bo_sb[:, c2:c2 + 1],
                        scalar2=None, op0=ADD)
                    nc.sync.dma_start(
                        out=out_d[c2 * 128:(c2 + 1) * 128, :], in_=ot)
                pending2.append(post_o)
                while len(pending2) > 2:
                    pending2.pop(0)()
            flush2()
    nc.compile()
    return nc


# ---------------------------------------------------------------------------
# host-side sharding
# ---------------------------------------------------------------------------

def band_mask(j):
    """[128, 32] multiplicative mask for the diagonal key tile band.

    Query col c of the 32-wide band maps to position j + 4*(32*kt + c);
    key row r maps to 128*kt + r: invalid iff r > j + 4c (kt cancels).
    """
    rr = np.arange(128)[:, None]
    cc = np.arange(32)[None, :]
    return (rr <= j + 4 * cc).astype(np.float32)


def make_in_maps(cfg, inputs):
    B, S, E, D, G = cfg["B"], cfg["S"], cfg["E"], cfg["D"], cfg["G"]
    NH, ET, NKT, QPC, GS = derived(cfg)
    x = np.asarray(inputs["x"], np.float32)
    shared = dict(
        Wq=np.ascontiguousarray(inputs["Wq"], np.float32),
        Wk=np.ascontiguousarray(inputs["Wk"], np.float32),
        Wv=np.ascontiguousarray(inputs["Wv"], np.float32),
        Wo=np.ascontiguousarray(inputs["Wo"], np.float32),
        bq_t=np.ascontiguousarray(
            np.asarray(inputs["bq"], np.float32).reshape(ET, 128).T),
        bk_t=np.ascontiguousarray(
            np.asarray(inputs["bk"], np.float32).reshape(G, 128).T),
        bv_b=np.ascontiguousarray(np.broadcast_to(
            np.asarray(inputs["bv"], np.float32).reshape(1, G * D),
            (128, G * D))),
        bo_t=np.ascontiguousarray(
            np.asarray(inputs["bo"], np.float32).reshape(ET, 128).T),
        gq_r=np.ascontiguousarray(
            np.asarray(inputs["gamma_q"], np.float32).reshape(1, 128)),
        gk_r=np.ascontiguousarray(
            np.asarray(inputs["gamma_k"], np.float32).reshape(1, 128)),
        ones1=np.ones((128, 1), np.float32),
    )
    xTb = [np.ascontiguousarray(x[b].T) for b in range(B)]
    in_maps = []
    for c in range(8):
        b, j = c // 4, c % 4
        m = dict(shared)
        m["xT"] = xTb[b]
        m["xq"] = np.ascontiguousarray(xTb[b][:, j::4])
        m["mask32"] = band_mask(j)
        in_maps.append(m)
    return in_maps, None


def assemble(cfg, results, perms):
    B, S, E = cfg["B"], cfg["S"], cfg["E"]
    out = np.empty((B, S, E), np.float32)
    for c in range(8):
        b, j = c // 4, c % 4
        out[b, j::4, :] = results[c]["outT"].T
    return out


_CACHE = {}


def kernel(**inputs):
    cfg = full_cfg()
    if "nc" not in _CACHE:
        _CACHE["nc"] = build_program(cfg)
    nc = _CACHE["nc"]
    in_maps, perms = make_in_maps(cfg, inputs)
    res = run_bass_kernel_spmd(nc, in_maps, list(range(8)))
    return assemble(cfg, res.results, perms)


# revision 37
# speedup vs baseline: 1.5086x; 1.3238x over previous
"""GQA attention block (RMSNorm-QK, causal, GQA) on 8 trn2 NeuronCores.

Strategy: strided sequence sharding, zero collectives. Core c handles batch
c//4 and queries at positions j::4 (j = c%4) of that batch, keys in natural
order. With stride-4 queries sorted ascending, causality is *uniform* across
cores: for key tile kt, exactly the query columns [32*kt, 512) are (at least
partially) valid, independent of j. Scores/exp/den/context matmuls are all
sliced to that suffix (~47% less attention work than full S), and the only
j-dependent data is a single [128, 32] diagonal band mask from the host.

All activations are feature-major ([feature, token]); V is produced directly
in [token, d] layout by swapping matmul operands (x tile stationary), so the
kernel has zero on-device transposes. Softmax needs no max subtraction
(RMS-normalized q,k bound |scores|/sqrt(D) <= sqrt(D)). Per-token softmax
denominators are rank-1 PE matmuls; 1/den uses the fast custom-DVE
reciprocal; RMS rsqrt runs on the ACT engine (one table set per phase).
"""

import math
import os
import numpy as np
from contextlib import ExitStack

import concourse.bass as bass
import concourse.mybir as mybir
import concourse.tile as tile
from concourse import bacc
from concourse.bass_utils import run_bass_kernel_spmd

F32 = mybir.dt.float32
F32R = mybir.dt.float32r
ADD = mybir.AluOpType.add
MULT = mybir.AluOpType.mult
EXP = mybir.ActivationFunctionType.Exp
SQRT = mybir.ActivationFunctionType.Sqrt

EPS = 1e-8


def full_cfg():
    return dict(B=2, S=2048, E=2048, D=128, G=2)


def derived(cfg):
    B, S, E, D, G = cfg["B"], cfg["S"], cfg["E"], cfg["D"], cfg["G"]
    NH = E // D            # query heads (16)
    ET = E // 128          # 128-row tiles of E (16)
    NKT = S // 128         # key tiles (16)
    QPC = S // 4           # query tokens per core (512)
    GS = NH // G           # heads per kv group (8)
    assert D == 128 and QPC == 512
    return NH, ET, NKT, QPC, GS


def build_program(cfg):
    B, S, E, D, G = cfg["B"], cfg["S"], cfg["E"], cfg["D"], cfg["G"]
    NH, ET, NKT, QPC, GS = derived(cfg)
    SCALE = 1.0 / math.sqrt(D)
    KC = 512               # token-column chunk width for K/V projections
    NKC = S // KC          # 4
    GRP = 3                # key tiles per exp group (3 PSUM banks)
    NGRP = (NKT + GRP - 1) // GRP  # 6 (last group has 1)
    sliced = os.environ.get("KERNEL_NO_SLICE", "0") != "1"

    def co(kt):            # first computed query column for key tile kt
        return 32 * kt if sliced else 0

    nc = bacc.Bacc()
    xT_d = nc.dram_tensor("xT", [E, S], F32, kind="ExternalInput")
    xq_d = nc.dram_tensor("xq", [E, QPC], F32, kind="ExternalInput")
    wq_d = nc.dram_tensor("Wq", [E, E], F32, kind="ExternalInput")
    wk_d = nc.dram_tensor("Wk", [E, G * D], F32, kind="ExternalInput")
    wv_d = nc.dram_tensor("Wv", [E, G * D], F32, kind="ExternalInput")
    wo_d = nc.dram_tensor("Wo", [E, E], F32, kind="ExternalInput")
    bq_d = nc.dram_tensor("bq_t", [128, ET], F32, kind="ExternalInput")
    bk_d = nc.dram_tensor("bk_t", [128, G], F32, kind="ExternalInput")
    bv_d = nc.dram_tensor("bv_b", [128, G * D], F32, kind="ExternalInput")
    bo_d = nc.dram_tensor("bo_t", [128, ET], F32, kind="ExternalInput")
    gq_d = nc.dram_tensor("gq_r", [1, 128], F32, kind="ExternalInput")
    gk_d = nc.dram_tensor("gk_r", [1, 128], F32, kind="ExternalInput")
    mask_d = nc.dram_tensor("mask32", [128, 32], F32, kind="ExternalInput")
    ones_d = nc.dram_tensor("ones1", [128, 1], F32, kind="ExternalInput")
    out_d = nc.dram_tensor("outT", [E, QPC], F32, kind="ExternalOutput")
    debug = os.environ.get("KERNEL_DEBUG_DEN", "0") == "1"
    if debug:
        dbgden_d = nc.dram_tensor("dbgden", [NH, QPC], F32, kind="ExternalOutput")
        dbgcx_d = nc.dram_tensor("dbgcx", [128, QPC], F32, kind="ExternalOutput")
        dbgq_d = nc.dram_tensor("dbgq", [128, QPC], F32, kind="ExternalOutput")
        dbgk_d = nc.dram_tensor("dbgk", [128, S], F32, kind="ExternalOutput")
        dbgv_d = nc.dram_tensor("dbgv", [128, NKT * 128], F32, kind="ExternalOutput")
        dbgsc_d = nc.dram_tensor("dbgsc", [NKT * 128, QPC], F32, kind="ExternalOutput")
        dbgpt_d = nc.dram_tensor("dbgpt", [NKT * 128, QPC], F32, kind="ExternalOutput")

    wq_r = wq_d.rearrange("(t p) c -> p t c", p=128)   # [128, ET, E]
    wk_r = wk_d.rearrange("(t p) c -> p t c", p=128)   # [128, ET, G*D]
    wv_r = wv_d.rearrange("(t p) c -> p t c", p=128)
    wo_r = wo_d.rearrange("(t p) c -> p t c", p=128)

    def r(ap):
        return ap if ap.dtype == F32R else ap.bitcast(F32R)

    with tile.TileContext(nc) as tc, ExitStack() as top:
        consts = top.enter_context(tc.tile_pool(name="consts", bufs=1))
        persist = top.enter_context(tc.tile_pool(name="persist", bufs=1))

        ones_col = consts.tile([128, 1], F32R)
        nc.sync.dma_start(out=ones_col, in_=ones_d[:, :].bitcast(F32R))
        ones_row = consts.tile([1, 128], F32)
        nc.vector.memset(ones_row, 1.0)
        eps_t = consts.tile([1, 1], F32)
        nc.vector.memset(eps_t, EPS)
        gq_sb = consts.tile([1, 128], F32)
        nc.sync.dma_start(out=gq_sb, in_=gq_d[:, :])
        gk_sb = consts.tile([1, 128], F32)
        nc.sync.dma_start(out=gk_sb, in_=gk_d[:, :])
        bq_sb = consts.tile([128, ET], F32)
        nc.sync.dma_start(out=bq_sb, in_=bq_d[:, :])
        bk_sb = consts.tile([128, G], F32)
        nc.sync.dma_start(out=bk_sb, in_=bk_d[:, :])
        bv_sb = consts.tile([128, G * D], F32)
        nc.sync.dma_start(out=bv_sb, in_=bv_d[:, :])
        bo_sb = consts.tile([128, ET], F32)
        nc.sync.dma_start(out=bo_sb, in_=bo_d[:, :])
        mask_sb = consts.tile([128, 32], F32R)
        nc.sync.dma_start(out=mask_sb, in_=mask_d[:, :].bitcast(F32R))

        ktn = [persist.tile([128, S], F32R, tag=f"ktn{g}", name=f"ktn{g}") for g in range(G)]
        vtok = [persist.tile([128, NKT, 128], F32R, tag=f"vtok{g}", name=f"vtok{g}") for g in range(G)]
        qtn = persist.tile([128, NH, QPC], F32R, tag="qtn")

        # ---------------- phase 1+2: projections ------------------------
        with ExitStack() as p12:
            wkvp = p12.enter_context(tc.tile_pool(name="wkv", bufs=1))
            xsp = p12.enter_context(tc.tile_pool(name="xs", bufs=18))
            xqp = p12.enter_context(tc.tile_pool(name="xqp", bufs=1))
            tmp = p12.enter_context(tc.tile_pool(name="tmp12", bufs=3))
            wqp = p12.enter_context(tc.tile_pool(name="wqs", bufs=2))
            pkv = p12.enter_context(tc.tile_pool(name="pkv", bufs=2, space="PSUM"))
            pv = p12.enter_context(tc.tile_pool(name="pv", bufs=4, space="PSUM"))
            pssq = p12.enter_context(tc.tile_pool(name="pssq", bufs=1, space="PSUM"))
            pbc = p12.enter_context(tc.tile_pool(name="pbc", bufs=1, space="PSUM"))

            wk_sb = wkvp.tile([128, ET, G * D], F32R, tag="wk")
            wv_sb = wkvp.tile([128, ET, G * D], F32R, tag="wv")

            pending = []

            def flush():
                while pending:
                    pending.pop(0)()

            for kc in range(NKC):
                xts = []
                for et in range(ET):
                    if kc == 0:
                        # interleave weight slices with the first x chunk so
                        # the K matmuls can start after ~3 transfers
                        nc.sync.dma_start(out=wk_sb[:, et, :],
                                          in_=wk_r[:, et, :].bitcast(F32R))
                        nc.sync.dma_start(out=wv_sb[:, et, :],
                                          in_=wv_r[:, et, :].bitcast(F32R))
                    xt = xsp.tile([128, KC], F32R, tag="xt")
                    nc.sync.dma_start(
                        out=xt, in_=xT_d[et * 128:(et + 1) * 128,
                                         kc * KC:(kc + 1) * KC].bitcast(F32R))
                    xts.append(xt)
                # K projection: [d, tok] layout
                acck = []
                for g in range(G):
                    acc = pkv.tile([128, KC], F32, tag="pkv", name="acck")
                    acck.append(acc)
                for et in range(ET):
                    for g in range(G):
                        nc.tensor.matmul(
                            acck[g],
                            lhsT=r(wk_sb[:, et, g * D:(g + 1) * D]),
                            rhs=r(xts[et]),
                            start=(et == 0), stop=(et == ET - 1))
                # V projection: [tok, d] layout (x tile stationary), bias folded
                accv = []
                for s in range(KC // 128):
                    acc = pv.tile([128, G * D], F32, tag="pv", name="accv")
                    accv.append(acc)
                    for et in range(ET):
                        nc.tensor.matmul(
                            acc,
                            lhsT=r(xts[et][:, s * 128:(s + 1) * 128]),
                            rhs=r(wv_sb[:, et, :]),
                            start=(et == 0), stop=(et == ET - 1))
                flush()

                def post_kv(kc=kc, acck=acck, accv=accv):
                    for g in range(G):
                        vb = tmp.tile([128, KC], F32, tag="vb", name="vb")
                        nc.vector.tensor_scalar(
                            out=vb, in0=acck[g], scalar1=bk_sb[:, g:g + 1],
                            scalar2=None, op0=ADD)
                        sq = tmp.tile([128, KC], F32R, tag="sq", name="sq")
                        nc.vector.tensor_tensor(out=sq, in0=vb, in1=vb, op=MULT)
                        ssq = pssq.tile([1, KC], F32, tag="ssq", name="ssq")
                        nc.tensor.matmul(ssq, lhsT=ones_col, rhs=r(sq),
                                         start=True, stop=True)
                        rms = tmp.tile([1, KC], F32, tag="rms", name="rms")
                        nc.scalar.activation(out=rms, in_=ssq, func=SQRT,
                                             scale=1.0 / D, bias=eps_t[:, :])
                        rinv = tmp.tile([1, KC], F32, tag="rinv", name="rinv")
                        nc.vector.reciprocal_approx_fast(out=rinv, in_=rms)
                        bc = pbc.tile([128, KC], F32, tag="bc", name="bc")
                        nc.tensor.matmul(bc, lhsT=gk_sb, rhs=rinv,
                                         start=True, stop=True)
                        nc.vector.tensor_tensor(
                            out=ktn[g][:, kc * KC:(kc + 1) * KC],
                            in0=vb, in1=bc, op=MULT)
                    for s in range(KC // 128):
                        kt = kc * (KC // 128) + s
                        for g in range(G):
                            nc.vector.tensor_tensor(
                                out=vtok[g][:, kt, :],
                                in0=accv[s][:, g * D:(g + 1) * D],
                                in1=bv_sb[:, g * D:(g + 1) * D], op=ADD)
                pending.append(post_kv)
            flush()

            # ---- phase 2: Q projection from host-gathered strided cols ----
            xq = []
            for et in range(ET):
                xt = xqp.tile([128, QPC], F32R, tag=f"xq{et}", name=f"xq{et}")
                nc.sync.dma_start(
                    out=xt, in_=xq_d[et * 128:(et + 1) * 128, :].bitcast(F32R))
                xq.append(xt)
            for qc in range(NH):
                wq_sb = wqp.tile([128, ET, 128], F32R, tag="wq", name="wq")
                nc.sync.dma_start(
                    out=wq_sb, in_=wq_r[:, :, qc * 128:(qc + 1) * 128].bitcast(F32R))
                acc = pkv.tile([128, QPC], F32, tag="pkv", name="qacc")
                for et in range(ET):
                    nc.tensor.matmul(acc, lhsT=r(wq_sb[:, et, :]),
                                     rhs=r(xq[et]),
                                     start=(et == 0), stop=(et == ET - 1))

                def post_q(qc=qc, acc=acc):
                    vb = tmp.tile([128, QPC], F32, tag="vb", name="qb")
                    nc.vector.tensor_scalar(
                        out=vb, in0=acc, scalar1=bq_sb[:, qc:qc + 1],
                        scalar2=None, op0=ADD)
                    sq = tmp.tile([128, QPC], F32R, tag="sq", name="qsq")
                    nc.vector.tensor_tensor(out=sq, in0=vb, in1=vb, op=MULT)
                    ssq = pssq.tile([1, QPC], F32, tag="ssq", name="qssq")
                    nc.tensor.matmul(ssq, lhsT=ones_col, rhs=r(sq),
                                     start=True, stop=True)
                    rms = tmp.tile([1, QPC], F32, tag="rms", name="qrms")
                    nc.scalar.activation(out=rms, in_=ssq, func=SQRT,
                                         scale=1.0 / D, bias=eps_t[:, :])
                    rinv = tmp.tile([1, QPC], F32, tag="rinv", name="qrinv")
                    nc.vector.reciprocal_approx_fast(out=rinv, in_=rms)
                    bc = pbc.tile([128, QPC], F32, tag="bc", name="qbc")
                    nc.tensor.matmul(bc, lhsT=gq_sb, rhs=rinv,
                                     start=True, stop=True)
                    nc.vector.tensor_tensor(out=qtn[:, qc, :], in0=vb,
                                            in1=bc, op=MULT)
                pending.append(post_q)
                if qc >= 1:
                    pending.pop(0)()
            flush()

        # ---------------- phase 3: attention + phase 4: out proj --------
        with ExitStack() as p34:
            ctxp = p34.enter_context(tc.tile_pool(name="ctxp", bufs=1))
            ctxt = ctxp.tile([128, ET, QPC], F32R, tag="ctxt", name="ctxt")
            ptp = p34.enter_context(tc.tile_pool(name="pt", bufs=1))
            smal = p34.enter_context(tc.tile_pool(name="smal", bufs=1))
            wop = p34.enter_context(tc.tile_pool(name="wos", bufs=3))
            osb = p34.enter_context(tc.tile_pool(name="osb", bufs=3))
            psc = p34.enter_context(tc.tile_pool(name="psc", bufs=1, space="PSUM"))
            pcx = p34.enter_context(tc.tile_pool(name="pcx", bufs=1, space="PSUM"))
            pdn = p34.enter_context(tc.tile_pool(name="pdn", bufs=1, space="PSUM"))

            cx = pcx.tile([128, QPC], F32, tag="cx", name="cx")
            dnb = pdn.tile([128, QPC], F32, tag="dnb", name="dnb")

            pending2 = []

            def flush2():
                while pending2:
                    pending2.pop(0)()

            if debug:
                nc.sync.dma_start(out=dbgq_d[:, :], in_=qtn[:, 0, :].bitcast(F32))
                nc.sync.dma_start(out=dbgk_d[:, :], in_=ktn[0][:, :].bitcast(F32))
                nc.sync.dma_start(
                    out=dbgv_d[:, :],
                    in_=vtok[0][:, :, :].bitcast(F32).rearrange("p a b -> p (a b)"))
            for h in range(NH):
                g_kv = h // GS
                for grp in range(NGRP):
                    kts = list(range(grp * GRP, min((grp + 1) * GRP, NKT)))
                    c0g = co(kts[0])
                    # fresh pool tiles per group: rotation inserts the WAR
                    # deps that make deferred emission safe
                    sct = psc.tile([128, GRP, QPC], F32, tag="sc", name="sct")
                    ptt = ptp.tile([128, GRP, QPC], F32R, tag="pt", name="ptt",
                                   bufs=4)
                    for i, kt in enumerate(kts):
                        nc.tensor.matmul(
                            sct[:, i, c0g:QPC],
                            lhsT=r(ktn[g_kv][:, kt * 128:(kt + 1) * 128]),
                            rhs=r(qtn[:, h, c0g:QPC]),
                            start=True, stop=True)

                    def post_grp(h=h, g_kv=g_kv, kts=kts, c0g=c0g,
                                 sct=sct, ptt=ptt):
                        n = len(kts)
                        if debug and h == 0:
                            for i, kt in enumerate(kts):
                                dsc = smal.tile([128, QPC], F32, tag="dsc",
                                                name="dsc", bufs=2)
                                nc.vector.tensor_copy(
                                    out=dsc[:, c0g:], in_=sct[:, i, c0g:])
                                nc.sync.dma_start(
                                    out=dbgsc_d[kt * 128:(kt + 1) * 128, c0g:],
                                    in_=dsc[:, c0g:])
                        nc.scalar.activation(
                            out=ptt[:, 0:n, c0g:QPC], in_=sct[:, 0:n, c0g:QPC],
                            func=EXP, scale=SCALE)
                        for i, kt in enumerate(kts):
                            nc.vector.tensor_tensor(
                                out=ptt[:, i, 32 * kt:32 * kt + 32],
                                in0=ptt[:, i, 32 * kt:32 * kt + 32],
                                in1=mask_sb, op=MULT)
                            if debug and h == 0:
                                nc.sync.dma_start(
                                    out=dbgpt_d[kt * 128:(kt + 1) * 128, c0g:],
                                    in_=ptt[:, i, c0g:].bitcast(F32))
                            nc.tensor.matmul(
                                dnb[0:1, co(kt):QPC], lhsT=ones_col,
                                rhs=r(ptt[:, i, co(kt):QPC]),
                                start=(kt == 0), stop=(kt == NKT - 1))
                            nc.tensor.matmul(
                                cx[:, co(kt):QPC],
                                lhsT=r(vtok[g_kv][:, kt, :]),
                                rhs=r(ptt[:, i, co(kt):QPC]),
                                start=(kt == 0), stop=(kt == NKT - 1))
                    pending2.append(post_grp)
                    while len(pending2) > 2:
                        pending2.pop(0)()

                def post_head(h=h):
                    if debug:
                        dsb = smal.tile([1, QPC], F32, tag=f"dbg{h}", name=f"dbg{h}")
                        nc.vector.tensor_copy(out=dsb, in_=dnb[0:1, :])
                        nc.sync.dma_start(out=dbgden_d[h:h + 1, :], in_=dsb)
                        if h == 0:
                            csb = smal.tile([128, QPC], F32, tag="dbgc", name="dbgc")
                            nc.vector.tensor_copy(out=csb, in_=cx)
                            nc.sync.dma_start(out=dbgcx_d[:, :], in_=csb)
                    rd = smal.tile([1, QPC], F32, tag="rd", name="rd", bufs=2)
                    nc.vector.reciprocal_approx_fast(out=rd, in_=dnb[0:1, :])
                    rdb = smal.tile([128, QPC], F32, tag="rdb", name="rdb",
                                    bufs=2)
                    nc.gpsimd.partition_broadcast(rdb, rd)
                    nc.vector.tensor_tensor(out=ctxt[:, h, :], in0=cx,
                                            in1=rdb, op=MULT)
                pending2.append(post_head)
            flush2()

            for c2 in range(ET):
                wo_sb = wop.tile([128, ET, 128], F32R, tag="wo", name="wo")
                nc.sync.dma_start(
                    out=wo_sb, in_=wo_r[:, :, c2 * 128:(c2 + 1) * 128].bitcast(F32R))
                acc = pcx.tile([128, QPC], F32, tag="cx", name="oacc")
                for ct in range(ET):
                    nc.tensor.matmul(acc, lhsT=r(wo_sb[:, ct, :]),
                                     rhs=r(ctxt[:, ct, :]),
                                     start=(ct == 0), stop=(ct == ET - 1))

                def post_o(c2=c2, acc=acc):
                    ot = osb.tile([128, QPC], F32, tag="ot", name="ot")
                    nc.vector.tensor_scalar(
                        out=ot, in0=acc, scalar1=bo_sb[:, c2:c2 + 1],
                        scalar2=None, op0=ADD)
                    nc.sync.dma_start(
                        out=out_d[c2 * 128:(c2 + 1) * 128, :], in_=ot)
                pending2.append(post_o)
                while len(pending2) > 2:
                    pending2.pop(0)()
            flush2()
    nc.compile()
    return nc


# ---------------------------------------------------------------------------
# host-side sharding
# ---------------------------------------------------------------------------

def band_mask(j):
    """[128, 32] multiplicative mask for the diagonal key tile band.

    Query col c of the 32-wide band maps to position j + 4*(32*kt + c);
    key row r maps to 128*kt + r: invalid iff r > j + 4c (kt cancels).
    """
    rr = np.arange(128)[:, None]
    cc = np.arange(32)[None, :]
    return (rr <= j + 4 * cc).astype(np.float32)


def make_in_maps(cfg, inputs):
    B, S, E, D, G = cfg["B"], cfg["S"], cfg["E"], cfg["D"], cfg["G"]
    NH, ET, NKT, QPC, GS = derived(cfg)
    x = np.asarray(inputs["x"], np.float32)
    shared = dict(
        Wq=np.ascontiguousarray(inputs["Wq"], np.float32),
        Wk=np.ascontiguousarray(inputs["Wk"], np.float32),
        Wv=np.ascontiguousarray(inputs["Wv"], np.float32),
        Wo=np.ascontiguousarray(inputs["Wo"], np.float32),
        bq_t=np.ascontiguousarray(
            np.asarray(inputs["bq"], np.float32).reshape(ET, 128).T),
        bk_t=np.ascontiguousarray(
            np.asarray(inputs["bk"], np.float32).reshape(G, 128).T),
        bv_b=np.ascontiguousarray(np.broadcast_to(
            np.asarray(inputs["bv"], np.float32).reshape(1, G * D),
            (128, G * D))),
        bo_t=np.ascontiguousarray(
            np.asarray(inputs["bo"], np.float32).reshape(ET, 128).T),
        gq_r=np.ascontiguousarray(
            np.asarray(inputs["gamma_q"], np.float32).reshape(1, 128)),
        gk_r=np.ascontiguousarray(
            np.asarray(inputs["gamma_k"], np.float32).reshape(1, 128)),
        ones1=np.ones((128, 1), np.float32),
    )
    xTb = [np.ascontiguousarray(x[b].T) for b in range(B)]
    in_maps = []
    for c in range(8):
        b, j = c // 4, c % 4
        m = dict(shared)
        m["xT"] = xTb[b]
        m["xq"] = np.ascontiguousarray(xTb[b][:, j::4])
        m["mask32"] = band_mask(j)
        in_maps.append(m)
    return in_maps, None


def assemble(cfg, results, perms):
    B, S, E = cfg["B"], cfg["S"], cfg["E"]
    out = np.empty((B, S, E), np.float32)
    for c in range(8):
        b, j = c // 4, c % 4
        out[b, j::4, :] = results[c]["outT"].T
    return out


_CACHE = {}


def kernel(**inputs):
    cfg = full_cfg()
    if "nc" not in _CACHE:
        _CACHE["nc"] = build_program(cfg)
    nc = _CACHE["nc"]
    in_maps, perms = make_in_maps(cfg, inputs)
    res = run_bass_kernel_spmd(nc, in_maps, list(range(8)))
    return assemble(cfg, res.results, perms)
